# revision 26
# baseline (speedup 1.0000x reference)
"""Batch-all triplet loss on 8 Trainium2 NeuronCores (Bass/Tile).

Math: with d = pairwise euclidean distance matrix of the B embeddings,
  loss = sum_{i,j,k valid} relu(d[i,j] - d[i,k] + margin) / (#positive + eps)
valid <=> i != j, labels[i] == labels[j], labels[i] != labels[k]
(the other distinctness constraints are implied by the label ones).

Sharding: anchors are grouped by class; each core hosts 2 classes in two
64-row blocks (data-driven gathers keep the single SPMD program uniform).

Per core, on device:
  prep:
  - bf16 matmul of the gathered anchors against all of X^T (row 513
    carries -0.5*||x_j||^2 as a bf16 hi + bf16 lo pair so the squared
    norm stays ~f32 exact) -> ACT Relu(-2*psum + sq_a) -> ACT Sqrt
    -> + mask (-32 shift on valid columns, +1e30 on own-class columns)
    gives d_in (bf16) = this core's rows of the distance matrix.
  - a small partner matmul the same way gives bias[r,t] = d(anchor_r,
    t-th member of r's class) + margin - 32 (bf16), stored next to d_in.
  pair loop (the B^3 work), NT tiles of 128 (anchor,positive) pairs:
  - PE replicates each pair's anchor row (+ its bias row) with a one-hot
    matmul into PSUM: rep[p, 0:640] = d_in[anchor(p), :],
    rep[p, 640:640+T] = bias[anchor(p), :].
  - DVE extracts the pair's bias: tensor_tensor_reduce(rep-bias-cols *
    one-hot-sel) -> bias_pair[p] (f32 scalar per partition).
  - ACT: activation(Relu, in=rep, scale=-1, bias=bias_pair, accum_out)
    = sum_k relu(p - n) for the 128 pairs at once.
  - DVE: tensor_scalar(is_lt, reduce-add) counts n < p (the > eps
    boundary is unreachable at fp granularity, so is_lt is exact).
  - invalid negatives contribute 0 (the +1e30 mask), padded pairs are
    all-zero rows with bias 0 and contribute 0 to both sums.
  - free-dim reduce -> [128, 2] stats DMA'd out; host adds them up.
"""

import numpy as np

import bass_rust
import concourse.bass as bass
import concourse.tile as tile
from concourse import mybir
from concourse.bass_utils import run_bass_kernel_spmd

N_CORES = 8
D_MODEL = 512
B_TOTAL = 640
MARGIN = 0.3
EPS = 1e-8
RB = 64  # rows per block (max class size the device path supports)
BIG = 1.0e30
SHIFT = 32.0  # d values live in ~[27.6, 37]; recentring helps bf16
F32 = mybir.dt.float32
BF16 = mybir.dt.bfloat16
NP_BF16 = mybir.dt.np(mybir.dt.bfloat16)

_PROGRAM_CACHE: dict = {}


def _split_multi_waits(nc):
    """This toolchain's walrus codegen supports only ONE sync-wait per
    instruction; Tile can emit several. Move the extra waits onto
    same-engine NoOps inserted immediately before the instruction."""
    for func in nc.m.functions:
        for block in func.blocks:
            out = []
            for inst in block.instructions:
                si = inst.sync_info
                waits = list(si.on_wait) if si else []
                if len(waits) > 1:
                    for j, w in enumerate(waits[:-1]):
                        nop = mybir.InstNoOp(
                            name=f"{inst.name}-wsplit{j}", ins=[], outs=[]
                        )
                        nop.engine = inst.engine
                        nop.sync_info = bass_rust.SyncInfo(on_wait=[w], on_update=[])
                        out.append(nop)
                    inst.sync_info = bass_rust.SyncInfo(
                        on_wait=[waits[-1]], on_update=list(si.on_update)
                    )
                out.append(inst)
            block.instructions = out


def _build_program(B: int, D: int, T: int, NT: int):
    """One SPMD program for all 8 cores; per-core behavior is data-driven."""
    nc = bass.Bass()

    # combo columns: [0:B) moving operand (X^T), [B:B+128) stationary
    # anchor gather, [B+128:B+128+2T) positive-partner gather. Packed
    # DRAM layout [128, 4, W] so one DMA covers all four K=128 chunks
    # with long per-partition descriptors; row 512 (hi) + its bf16
    # residual live in `aug` ([2, W]).
    W = B + 128 + 2 * T
    combo = nc.declare_dram_parameter("combo", [128, 4 * W], BF16, isOutput=False)
    aug = nc.declare_dram_parameter("aug", [2, W], BF16, isOutput=False)
    sq_a = nc.declare_dram_parameter("sq_a", [128, 1], F32, isOutput=False)
    m_in = nc.declare_dram_parameter("m_in", [128, B], BF16, isOutput=False)
    onehot = nc.declare_dram_parameter("onehot", [128, NT * 128], BF16, isOutput=False)
    sel = nc.declare_dram_parameter("sel", [128, NT * 2 * T], BF16, isOutput=False)
    out_d = nc.declare_dram_parameter("out", [2, 128], F32, isOutput=True)

    WR = B + 2 * T  # replicated tile: negatives + bias hi + bias lo columns

    with tile.TileContext(nc) as tc:
        with (
            tc.tile_pool(name="const", bufs=1) as const,
            tc.tile_pool(name="work", bufs=1) as work,
        ):
            # tiny inputs first, then the matmul operands 8-way split so
            # the K-chunk pipeline starts as early as possible
            taug_hi = const.tile([1, W], BF16)
            nc.sync.dma_start(out=taug_hi, in_=aug[0:1, :])
            taug_lo = const.tile([1, W], BF16)
            nc.sync.dma_start(out=taug_lo, in_=aug[1:2, :])
            sqa = const.tile([128, 1], F32)
            nc.sync.dma_start(out=sqa, in_=sq_a[:, :])
            # preload the ACT table set while DMAs run
            warm = const.tile([1, 8], F32)
            nc.vector.memset(warm, 1.0)
            nc.scalar.activation(out=warm, in_=warm,
                                 func=mybir.ActivationFunctionType.Relu)
            nc.scalar.activation(out=warm, in_=warm,
                                 func=mybir.ActivationFunctionType.Sqrt)
            ktile = const.tile([128, 4, W], BF16)
            combo3 = combo[:, :].rearrange("p (c w) -> p c w", c=4)
            wh = W // 2
            for ki in range(4):
                nc.sync.dma_start(out=ktile[:, ki, 0:wh], in_=combo3[:, ki, 0:wh])
                nc.sync.dma_start(out=ktile[:, ki, wh:W], in_=combo3[:, ki, wh:W])
            minm = const.tile([128, B], BF16)
            nc.sync.dma_start(out=minm, in_=m_in[:, :])
            t_oh = const.tile([128, NT * 128], BF16)
            ohq = (NT + 3) // 4 * 128
            for c in range(4):
                c0, c1 = c * ohq, min((c + 1) * ohq, NT * 128)
                if c0 < c1:
                    nc.sync.dma_start(out=t_oh[:, c0:c1], in_=onehot[:, c0:c1])
            t_sel = const.tile([128, NT * 2 * T], BF16)
            sq2 = (NT + 1) // 2 * 2 * T
            for c in range(2):
                c0, c1 = c * sq2, min((c + 1) * sq2, NT * 2 * T)
                if c0 < c1:
                    nc.sync.dma_start(out=t_sel[:, c0:c1], in_=sel[:, c0:c1])
            ident = nc.declare_dram_parameter("ident", [128, 128], F32, isOutput=False)
            t_id = const.tile([128, 128], F32)
            nc.sync.dma_start(out=t_id, in_=ident[:, :])

            L0, L1 = B, B + 128  # stationary (anchor) columns
            P0 = B + 128  # partner columns

            rhs_c = work.tile([128, WR], BF16)  # [d_in | bias] combined

            with tc.tile_pool(name="psum_prep", bufs=1, space="PSUM") as psp:
                # ---- distance rows: PSUM g = dot(x_a, x_j) - 0.5*sq_j ----
                g = psp.tile([128, B], F32)
                for n0, n1 in [(0, 512), (512, B)]:
                    for ki in range(4):
                        nc.tensor.matmul(
                            g[:, n0:n1],
                            ktile[:, ki, L0:L1],
                            ktile[:, ki, n0:n1],
                            start=(ki == 0),
                            stop=False,
                        )
                    nc.tensor.matmul(
                        g[:, n0:n1], taug_hi[:, L0:L1], taug_hi[:, n0:n1],
                        start=False, stop=False,
                    )
                    nc.tensor.matmul(
                        g[:, n0:n1], taug_hi[:, L0:L1], taug_lo[:, n0:n1],
                        start=False, stop=True,
                    )
                dsq = work.tile([128, B], F32)
                nc.scalar.activation(
                    out=dsq, in_=g, func=mybir.ActivationFunctionType.Relu,
                    bias=sqa, scale=-2.0,
                )
                dmat = work.tile([128, B], F32)
                nc.scalar.activation(
                    out=dmat, in_=dsq, func=mybir.ActivationFunctionType.Sqrt,
                )
                # d_in = d - 32 on valid columns, +1e30 on own-class cols
                nc.vector.tensor_add(rhs_c[:, 0:B], dmat, minm)

                # ---- positive-pair distances: bias[r, t] ----
                pb = psp.tile([128, T], F32)
                for blk in range(2):
                    r0, r1 = blk * RB, (blk + 1) * RB
                    c0, c1 = P0 + blk * T, P0 + (blk + 1) * T
                    for ki in range(4):
                        nc.tensor.matmul(
                            pb[r0:r1, :],
                            ktile[:, ki, L0 + r0 : L0 + r1],
                            ktile[:, ki, c0:c1],
                            start=(ki == 0),
                            stop=False,
                        )
                    nc.tensor.matmul(
                        pb[r0:r1, :], taug_hi[:, L0 + r0 : L0 + r1],
                        taug_hi[:, c0:c1], start=False, stop=False,
                    )
                    nc.tensor.matmul(
                        pb[r0:r1, :], taug_hi[:, L0 + r0 : L0 + r1],
                        taug_lo[:, c0:c1], start=False, stop=True,
                    )
                bsq = work.tile([128, T], F32)
                nc.scalar.activation(
                    out=bsq, in_=pb, func=mybir.ActivationFunctionType.Relu,
                    bias=sqa, scale=-2.0,
                )
                bd = work.tile([128, T], F32)
                nc.scalar.activation(
                    out=bd, in_=bsq, func=mybir.ActivationFunctionType.Sqrt,
                )
                # bias = d_pos + margin - 32, carried as bf16 hi + lo so
                # the pair threshold stays ~f32-exact (avoids bf16 grid
                # ties between bias and distances, which bias the count)
                bias_f = work.tile([128, T], F32)
                nc.vector.tensor_scalar(
                    out=bias_f, in0=bd,
                    scalar1=float(MARGIN - SHIFT), scalar2=None,
                    op0=mybir.AluOpType.add,
                )
                nc.vector.tensor_copy(rhs_c[:, B : B + T], bias_f)
                nc.vector.tensor_sub(
                    rhs_c[:, B + T : B + 2 * T], bias_f, rhs_c[:, B : B + T]
                )

            # ---- pair loop ----
            with (
                tc.tile_pool(name="psum_loop", bufs=3, space="PSUM") as psl,
                tc.tile_pool(name="psum_acc", bufs=1, space="PSUM") as psa,
                tc.tile_pool(name="scr_a", bufs=2) as scr_a,
                tc.tile_pool(name="scr_v", bufs=2) as scr_v,
                tc.tile_pool(name="bp", bufs=3) as bpp,
            ):
                sum_ps = psa.tile([128, NT], F32)
                cnt_cols = work.tile([128, NT], F32)
                for tau in range(NT):
                    rep = psl.tile([128, WR], F32, tag="rep")
                    oh = t_oh[:, tau * 128 : (tau + 1) * 128]
                    for n0, n1 in [(0, 512), (512, WR)]:
                        nc.tensor.matmul(
                            rep[:, n0:n1], oh, rhs_c[:, n0:n1],
                            start=True, stop=True,
                        )
                    bias_p = bpp.tile([128, 1], F32, tag="bias_p")
                    ttr_o = scr_v.tile([128, 2 * T], F32, tag="ttr")
                    nc.vector.scalar_tensor_tensor(
                        out=ttr_o, in0=rep[:, B : B + 2 * T], scalar=1.0,
                        in1=t_sel[:, tau * 2 * T : (tau + 1) * 2 * T],
                        op0=mybir.AluOpType.mult, op1=mybir.AluOpType.mult,
                        accum_out=bias_p,
                    )
                    o1 = scr_a.tile([128, B], F32, tag="oact")
                    nc.scalar.activation(
                        out=o1, in_=rep[:, 0:B],
                        func=mybir.ActivationFunctionType.Relu,
                        bias=bias_p, scale=-1.0,
                        accum_out=sum_ps[:, tau : tau + 1],
                    )
                    o2 = scr_v.tile([128, B], F32, tag="ocnt")
                    nc.vector.tensor_scalar(
                        out=o2, in0=rep[:, 0:B],
                        scalar1=bias_p, scalar2=0.0,
                        op0=mybir.AluOpType.is_lt, op1=mybir.AluOpType.add,
                        accum_out=cnt_cols[:, tau : tau + 1],
                    )

                # ---- free-dim reduce; transpose so the out DMA is two
                # long descriptors instead of 128 8-byte ones ----
                stat = work.tile([128, 2], F32)
                nc.vector.tensor_reduce(
                    out=stat[:, 0:1], in_=sum_ps, axis=mybir.AxisListType.X,
                    op=mybir.AluOpType.add,
                )
                nc.vector.tensor_reduce(
                    out=stat[:, 1:2], in_=cnt_cols, axis=mybir.AxisListType.X,
                    op=mybir.AluOpType.add,
                )
                stat_t = psa.tile([2, 128], F32, tag="stat_t")
                nc.tensor.transpose(stat_t, stat, t_id)
                stat_s = work.tile([2, 128], F32)
                nc.vector.tensor_copy(stat_s, stat_t)
                nc.sync.dma_start(out=out_d[:, :], in_=stat_s)

    _split_multi_waits(nc)
    return nc


def _schedule(labels: np.ndarray):
    """Group anchors by class, pair classes onto cores (big with small)."""
    vals, counts = np.unique(labels, return_counts=True)
    classes = [np.nonzero(labels == v)[0] for v in vals]
    order = np.argsort(-counts, kind="stable")
    classes = [classes[i] for i in order]
    sizes = [len(c) for c in classes]
    if len(classes) > 2 * N_CORES or max(sizes) > RB:
        return None  # device path infeasible for this label layout
    while len(classes) < 2 * N_CORES:
        classes.append(np.zeros((0,), dtype=np.int64))
    blocks = []
    for i in range(N_CORES):
        blocks.append((classes[i], classes[2 * N_CORES - 1 - i]))
    T = max(1, max(len(c) for c, _ in blocks))
    npairs = [len(a) * (len(a) - 1) + len(b) * (len(b) - 1) for a, b in blocks]
    NT = max(1, (max(npairs) + 127) // 128)
    return blocks, T, NT


def _host_fallback(X: np.ndarray, labels: np.ndarray) -> np.float32:
    """Exact numpy implementation (only for label layouts the device
    schedule cannot represent — cannot occur for randint(0,16) labels)."""
    Xd = X.astype(np.float64)
    dot = Xd @ Xd.T
    sq = np.diag(dot).copy()
    dm = np.maximum(sq[None, :] - 2.0 * dot + sq[:, None], 0.0)
    zero = dm == 0.0
    dm = np.sqrt(dm + zero * EPS) * (1.0 - zero)
    total = 0.0
    npos = 0
    B = len(labels)
    for i in range(B):
        pos = (labels == labels[i]) & (np.arange(B) != i)
        neg = labels != labels[i]
        p = dm[i, pos] + MARGIN
        n = dm[i, neg]
        tl = np.maximum(p[:, None] - n[None, :], 0.0)
        total += tl.sum()
        npos += (tl > EPS).sum()
    return np.float32(total / (npos + EPS))


def _make_in_maps(X: np.ndarray, lab: np.ndarray, blocks, T: int, NT: int):
    B, D = X.shape
    sq = (X.astype(np.float64) ** 2).sum(axis=1).astype(np.float32)
    W = B + 128 + 2 * T

    XT = np.ascontiguousarray(X.T)
    in_maps = []
    for core in range(N_CORES):
        cls_a, cls_b = blocks[core]
        row_idx = np.zeros(128, dtype=np.int64)
        for blk, cls in enumerate((cls_a, cls_b)):
            m = len(cls)
            r0 = blk * RB
            if m:
                row_idx[r0 : r0 + m] = cls
                row_idx[r0 + m : r0 + RB] = cls[0]

        par_idx = np.zeros(2 * T, dtype=np.int64)
        for blk, cls in enumerate((cls_a, cls_b)):
            m = len(cls)
            c0 = blk * T
            if m:
                par_idx[c0 : c0 + m] = cls

        combo = np.empty((D + 1, W), dtype=np.float32)
        combo[:D, :B] = XT
        combo[D, :B] = -0.5 * sq
        combo[:D, B : B + 128] = XT[:, row_idx]
        combo[D, B : B + 128] = 1.0
        combo[:D, B + 128 :] = XT[:, par_idx]
        combo[D, B + 128 :] = -0.5 * sq[par_idx]

        combo_hi = combo.astype(NP_BF16)
        lo = (combo[D, :] - combo_hi[D, :].astype(np.float32)).astype(NP_BF16)
        aug = np.stack([combo_hi[D], lo])  # [2, W]
        # packed [128, 4, W]: partition p, chunk c -> K-row c*128+p
        packed = np.ascontiguousarray(
            combo_hi[:D].reshape(4, 128, W).transpose(1, 0, 2)
        ).reshape(128, 4 * W)

        sq_a = sq[row_idx].reshape(128, 1).astype(np.float32)

        # -SHIFT on valid columns, +BIG on own-class columns (incl. self)
        m_in = np.full((128, B), -SHIFT, dtype=np.float32)
        for blk, cls in enumerate((cls_a, cls_b)):
            if len(cls):
                own = np.isin(lab, lab[cls[0]])
                m_in[blk * RB : (blk + 1) * RB, own] = BIG

        # pair tables: one-hot anchor pick and bias-column select
        onehot = np.zeros((128, NT * 128), dtype=NP_BF16)
        selm = np.zeros((128, NT * 2 * T), dtype=NP_BF16)
        p = 0
        for blk, cls in enumerate((cls_a, cls_b)):
            m = len(cls)
            r0 = blk * RB
            for i in range(m):
                for t in range(m):
                    if t == i:
                        continue
                    tau, q = divmod(p, 128)
                    onehot[r0 + i, tau * 128 + q] = 1.0
                    selm[q, tau * 2 * T + t] = 1.0
                    selm[q, tau * 2 * T + T + t] = 1.0
                    p += 1
        assert p <= NT * 128

        in_maps.append(
            {
                "combo": packed,
                "aug": aug,
                "sq_a": sq_a,
                "m_in": m_in.astype(NP_BF16),
                "onehot": onehot,
                "sel": selm,
                "ident": np.eye(128, dtype=np.float32),
            }
        )
    return in_maps


def kernel(embeddings: np.ndarray, labels: np.ndarray) -> np.ndarray:
    X = np.ascontiguousarray(np.asarray(embeddings), dtype=np.float32)
    lab = np.asarray(labels).astype(np.int64)
    B, D = X.shape
    assert B == B_TOTAL and D == D_MODEL, (B, D)

    sched = _schedule(lab)
    if sched is None:
        return _host_fallback(X, lab)
    blocks, T, NT = sched
    in_maps = _make_in_maps(X, lab, blocks, T, NT)

    key = (B, D, T, NT)
    nc = _PROGRAM_CACHE.get(key)
    if nc is None:
        nc = _build_program(B, D, T, NT)
        _PROGRAM_CACHE[key] = nc

    res = run_bass_kernel_spmd(nc, in_maps, core_ids=list(range(N_CORES)))
    total_sum = 0.0
    total_cnt = 0.0
    for r in res.results:
        o = np.asarray(r["out"], dtype=np.float64)
        total_sum += o[0].sum()
        total_cnt += o[1].sum()
    return np.float32(total_sum / (total_cnt + EPS))


# revision 27
# speedup vs baseline: 1.0424x; 1.0424x over previous
"""Batch-all triplet loss on 8 Trainium2 NeuronCores (Bass/Tile).

Math: with d = pairwise euclidean distance matrix of the B embeddings,
  loss = sum_{i,j,k valid} relu(d[i,j] - d[i,k] + margin) / (#positive + eps)
valid <=> i != j, labels[i] == labels[j], labels[i] != labels[k]
(the other distinctness constraints are implied by the label ones).

Sharding: anchors are grouped by class; each core hosts 2 classes in two
64-row blocks (data-driven gathers keep the single SPMD program uniform).

Per core, on device:
  prep:
  - bf16 matmul of the gathered anchors against all of X^T (row 513
    carries -0.5*||x_j||^2 as a bf16 hi + bf16 lo pair so the squared
    norm stays ~f32 exact) -> ACT Relu(-2*psum + sq_a) -> ACT Sqrt
    -> + mask (-32 shift on valid columns, +1e30 on own-class columns)
    gives d_in (bf16) = this core's rows of the distance matrix.
  - a small partner matmul the same way gives bias[r,t] = d(anchor_r,
    t-th member of r's class) + margin - 32 (bf16), stored next to d_in.
  pair loop (the B^3 work), NT tiles of 128 (anchor,positive) pairs:
  - PE replicates each pair's anchor row (+ its bias row) with a one-hot
    matmul into PSUM: rep[p, 0:640] = d_in[anchor(p), :],
    rep[p, 640:640+T] = bias[anchor(p), :].
  - DVE extracts the pair's bias: tensor_tensor_reduce(rep-bias-cols *
    one-hot-sel) -> bias_pair[p] (f32 scalar per partition).
  - ACT: activation(Relu, in=rep, scale=-1, bias=bias_pair, accum_out)
    = sum_k relu(p - n) for the 128 pairs at once.
  - DVE: tensor_scalar(is_lt, reduce-add) counts n < p (the > eps
    boundary is unreachable at fp granularity, so is_lt is exact).
  - invalid negatives contribute 0 (the +1e30 mask), padded pairs are
    all-zero rows with bias 0 and contribute 0 to both sums.
  - free-dim reduce -> [128, 2] stats DMA'd out; host adds them up.
"""

import numpy as np

import bass_rust
import concourse.bass as bass
import concourse.tile as tile
from concourse import mybir
from concourse.bass_utils import run_bass_kernel_spmd

N_CORES = 8
D_MODEL = 512
B_TOTAL = 640
MARGIN = 0.3
EPS = 1e-8
RB = 64  # rows per block (max class size the device path supports)
BIG = 1.0e30
SHIFT = 32.0  # d values live in ~[27.6, 37]; recentring helps bf16
F32 = mybir.dt.float32
BF16 = mybir.dt.bfloat16
NP_BF16 = mybir.dt.np(mybir.dt.bfloat16)

_PROGRAM_CACHE: dict = {}


def _split_multi_waits(nc):
    """This toolchain's walrus codegen supports only ONE sync-wait per
    instruction; Tile can emit several. Move the extra waits onto
    same-engine NoOps inserted immediately before the instruction."""
    for func in nc.m.functions:
        for block in func.blocks:
            out = []
            for inst in block.instructions:
                si = inst.sync_info
                waits = list(si.on_wait) if si else []
                if len(waits) > 1:
                    for j, w in enumerate(waits[:-1]):
                        nop = mybir.InstNoOp(
                            name=f"{inst.name}-wsplit{j}", ins=[], outs=[]
                        )
                        nop.engine = inst.engine
                        nop.sync_info = bass_rust.SyncInfo(on_wait=[w], on_update=[])
                        out.append(nop)
                    inst.sync_info = bass_rust.SyncInfo(
                        on_wait=[waits[-1]], on_update=list(si.on_update)
                    )
                out.append(inst)
            block.instructions = out


def _build_program(B: int, D: int, T: int, NT: int):
    """One SPMD program for all 8 cores; per-core behavior is data-driven."""
    nc = bass.Bass()

    # combo columns: [0:B) moving operand (X^T), [B:B+128) stationary
    # anchor gather, [B+128:B+128+2T) positive-partner gather. Packed
    # DRAM layout [128, 4, W] so one DMA covers all four K=128 chunks
    # with long per-partition descriptors; row 512 (hi) + its bf16
    # residual live in `aug` ([2, W]).
    W = B + 128 + 2 * T
    combo = nc.declare_dram_parameter("combo", [128, 4 * W], BF16, isOutput=False)
    aug = nc.declare_dram_parameter("aug", [2, W], BF16, isOutput=False)
    sq_a = nc.declare_dram_parameter("sq_a", [128, 1], F32, isOutput=False)
    m_in = nc.declare_dram_parameter("m_in", [128, B], BF16, isOutput=False)
    onehot = nc.declare_dram_parameter("onehot", [128, NT * 128], BF16, isOutput=False)
    sel = nc.declare_dram_parameter("sel", [128, NT * 2 * T], BF16, isOutput=False)
    out_d = nc.declare_dram_parameter("out", [2, 128], F32, isOutput=True)

    WR = B + 2 * T  # replicated tile: negatives + bias hi + bias lo columns

    with tile.TileContext(nc) as tc:
        with (
            tc.tile_pool(name="const", bufs=1) as const,
            tc.tile_pool(name="work", bufs=1) as work,
        ):
            # tiny inputs first, then the matmul operands 8-way split so
            # the K-chunk pipeline starts as early as possible
            taug_hi = const.tile([1, W], BF16)
            nc.sync.dma_start(out=taug_hi, in_=aug[0:1, :])
            taug_lo = const.tile([1, W], BF16)
            nc.sync.dma_start(out=taug_lo, in_=aug[1:2, :])
            sqa = const.tile([128, 1], F32)
            nc.sync.dma_start(out=sqa, in_=sq_a[:, :])
            # preload the ACT table set while DMAs run
            warm = const.tile([1, 8], F32)
            nc.vector.memset(warm, 1.0)
            nc.scalar.activation(out=warm, in_=warm,
                                 func=mybir.ActivationFunctionType.Relu)
            nc.scalar.activation(out=warm, in_=warm,
                                 func=mybir.ActivationFunctionType.Sqrt)
            ktile = const.tile([128, 4, W], BF16)
            combo3 = combo[:, :].rearrange("p (c w) -> p c w", c=4)
            for ki in range(4):
                nc.sync.dma_start(out=ktile[:, ki, :], in_=combo3[:, ki, :])
            minm = const.tile([128, B], BF16)
            nc.sync.dma_start(out=minm, in_=m_in[:, :])
            t_oh = const.tile([128, NT * 128], BF16)
            ohq = (NT + 3) // 4 * 128
            for c in range(4):
                c0, c1 = c * ohq, min((c + 1) * ohq, NT * 128)
                if c0 < c1:
                    nc.sync.dma_start(out=t_oh[:, c0:c1], in_=onehot[:, c0:c1])
            t_sel = const.tile([128, NT * 2 * T], BF16)
            sq2 = (NT + 1) // 2 * 2 * T
            for c in range(2):
                c0, c1 = c * sq2, min((c + 1) * sq2, NT * 2 * T)
                if c0 < c1:
                    nc.sync.dma_start(out=t_sel[:, c0:c1], in_=sel[:, c0:c1])
            ident = nc.declare_dram_parameter("ident", [128, 128], F32, isOutput=False)
            t_id = const.tile([128, 128], F32)
            nc.sync.dma_start(out=t_id, in_=ident[:, :])

            L0, L1 = B, B + 128  # stationary (anchor) columns
            P0 = B + 128  # partner columns

            rhs_c = work.tile([128, WR], BF16)  # [d_in | bias] combined

            with tc.tile_pool(name="psum_prep", bufs=1, space="PSUM") as psp:
                # ---- distance rows: PSUM g = dot(x_a, x_j) - 0.5*sq_j ----
                g = psp.tile([128, B], F32)
                for n0, n1 in [(0, 512), (512, B)]:
                    for ki in range(4):
                        nc.tensor.matmul(
                            g[:, n0:n1],
                            ktile[:, ki, L0:L1],
                            ktile[:, ki, n0:n1],
                            start=(ki == 0),
                            stop=False,
                        )
                    nc.tensor.matmul(
                        g[:, n0:n1], taug_hi[:, L0:L1], taug_hi[:, n0:n1],
                        start=False, stop=False,
                    )
                    nc.tensor.matmul(
                        g[:, n0:n1], taug_hi[:, L0:L1], taug_lo[:, n0:n1],
                        start=False, stop=True,
                    )
                dsq = work.tile([128, B], F32)
                nc.scalar.activation(
                    out=dsq, in_=g, func=mybir.ActivationFunctionType.Relu,
                    bias=sqa, scale=-2.0,
                )
                dmat = work.tile([128, B], F32)
                nc.scalar.activation(
                    out=dmat, in_=dsq, func=mybir.ActivationFunctionType.Sqrt,
                )
                # d_in = d - 32 on valid columns, +1e30 on own-class cols
                nc.vector.tensor_add(rhs_c[:, 0:B], dmat, minm)

                # ---- positive-pair distances: bias[r, t] ----
                pb = psp.tile([128, T], F32)
                for blk in range(2):
                    r0, r1 = blk * RB, (blk + 1) * RB
                    c0, c1 = P0 + blk * T, P0 + (blk + 1) * T
                    for ki in range(4):
                        nc.tensor.matmul(
                            pb[r0:r1, :],
                            ktile[:, ki, L0 + r0 : L0 + r1],
                            ktile[:, ki, c0:c1],
                            start=(ki == 0),
                            stop=False,
                        )
                    nc.tensor.matmul(
                        pb[r0:r1, :], taug_hi[:, L0 + r0 : L0 + r1],
                        taug_hi[:, c0:c1], start=False, stop=False,
                    )
                    nc.tensor.matmul(
                        pb[r0:r1, :], taug_hi[:, L0 + r0 : L0 + r1],
                        taug_lo[:, c0:c1], start=False, stop=True,
                    )
                bsq = work.tile([128, T], F32)
                nc.scalar.activation(
                    out=bsq, in_=pb, func=mybir.ActivationFunctionType.Relu,
                    bias=sqa, scale=-2.0,
                )
                bd = work.tile([128, T], F32)
                nc.scalar.activation(
                    out=bd, in_=bsq, func=mybir.ActivationFunctionType.Sqrt,
                )
                # bias = d_pos + margin - 32, carried as bf16 hi + lo so
                # the pair threshold stays ~f32-exact (avoids bf16 grid
                # ties between bias and distances, which bias the count)
                bias_f = work.tile([128, T], F32)
                nc.vector.tensor_scalar(
                    out=bias_f, in0=bd,
                    scalar1=float(MARGIN - SHIFT), scalar2=None,
                    op0=mybir.AluOpType.add,
                )
                nc.vector.tensor_copy(rhs_c[:, B : B + T], bias_f)
                nc.vector.tensor_sub(
                    rhs_c[:, B + T : B + 2 * T], bias_f, rhs_c[:, B : B + T]
                )

            # ---- pair loop ----
            with (
                tc.tile_pool(name="psum_loop", bufs=3, space="PSUM") as psl,
                tc.tile_pool(name="psum_acc", bufs=1, space="PSUM") as psa,
                tc.tile_pool(name="scr_a", bufs=2) as scr_a,
                tc.tile_pool(name="scr_v", bufs=2) as scr_v,
                tc.tile_pool(name="bp", bufs=3) as bpp,
            ):
                sum_ps = psa.tile([128, NT], F32)
                cnt_cols = work.tile([128, NT], F32)
                for tau in range(NT):
                    rep = psl.tile([128, WR], F32, tag="rep")
                    oh = t_oh[:, tau * 128 : (tau + 1) * 128]
                    for n0, n1 in [(0, 512), (512, WR)]:
                        nc.tensor.matmul(
                            rep[:, n0:n1], oh, rhs_c[:, n0:n1],
                            start=True, stop=True,
                        )
                    bias_p = bpp.tile([128, 1], F32, tag="bias_p")
                    ttr_o = scr_v.tile([128, 2 * T], F32, tag="ttr")
                    nc.vector.scalar_tensor_tensor(
                        out=ttr_o, in0=rep[:, B : B + 2 * T], scalar=1.0,
                        in1=t_sel[:, tau * 2 * T : (tau + 1) * 2 * T],
                        op0=mybir.AluOpType.mult, op1=mybir.AluOpType.mult,
                        accum_out=bias_p,
                    )
                    o1 = scr_a.tile([128, B], F32, tag="oact")
                    nc.scalar.activation(
                        out=o1, in_=rep[:, 0:B],
                        func=mybir.ActivationFunctionType.Relu,
                        bias=bias_p, scale=-1.0,
                        accum_out=sum_ps[:, tau : tau + 1],
                    )
                    o2 = scr_v.tile([128, B], F32, tag="ocnt")
                    nc.vector.tensor_scalar(
                        out=o2, in0=rep[:, 0:B],
                        scalar1=bias_p, scalar2=0.0,
                        op0=mybir.AluOpType.is_lt, op1=mybir.AluOpType.add,
                        accum_out=cnt_cols[:, tau : tau + 1],
                    )

                # ---- free-dim reduce; transpose so the out DMA is two
                # long descriptors instead of 128 8-byte ones ----
                stat = work.tile([128, 2], F32)
                nc.vector.tensor_reduce(
                    out=stat[:, 0:1], in_=sum_ps, axis=mybir.AxisListType.X,
                    op=mybir.AluOpType.add,
                )
                nc.vector.tensor_reduce(
                    out=stat[:, 1:2], in_=cnt_cols, axis=mybir.AxisListType.X,
                    op=mybir.AluOpType.add,
                )
                stat_t = psa.tile([2, 128], F32, tag="stat_t")
                nc.tensor.transpose(stat_t, stat, t_id)
                stat_s = work.tile([2, 128], F32)
                nc.vector.tensor_copy(stat_s, stat_t)
                nc.sync.dma_start(out=out_d[:, :], in_=stat_s)

    _split_multi_waits(nc)
    return nc


def _schedule(labels: np.ndarray):
    """Group anchors by class, pair classes onto cores (big with small)."""
    vals, counts = np.unique(labels, return_counts=True)
    classes = [np.nonzero(labels == v)[0] for v in vals]
    order = np.argsort(-counts, kind="stable")
    classes = [classes[i] for i in order]
    sizes = [len(c) for c in classes]
    if len(classes) > 2 * N_CORES or max(sizes) > RB:
        return None  # device path infeasible for this label layout
    while len(classes) < 2 * N_CORES:
        classes.append(np.zeros((0,), dtype=np.int64))
    blocks = []
    for i in range(N_CORES):
        blocks.append((classes[i], classes[2 * N_CORES - 1 - i]))
    T = max(1, max(len(c) for c, _ in blocks))
    npairs = [len(a) * (len(a) - 1) + len(b) * (len(b) - 1) for a, b in blocks]
    NT = max(1, (max(npairs) + 127) // 128)
    return blocks, T, NT


def _host_fallback(X: np.ndarray, labels: np.ndarray) -> np.float32:
    """Exact numpy implementation (only for label layouts the device
    schedule cannot represent — cannot occur for randint(0,16) labels)."""
    Xd = X.astype(np.float64)
    dot = Xd @ Xd.T
    sq = np.diag(dot).copy()
    dm = np.maximum(sq[None, :] - 2.0 * dot + sq[:, None], 0.0)
    zero = dm == 0.0
    dm = np.sqrt(dm + zero * EPS) * (1.0 - zero)
    total = 0.0
    npos = 0
    B = len(labels)
    for i in range(B):
        pos = (labels == labels[i]) & (np.arange(B) != i)
        neg = labels != labels[i]
        p = dm[i, pos] + MARGIN
        n = dm[i, neg]
        tl = np.maximum(p[:, None] - n[None, :], 0.0)
        total += tl.sum()
        npos += (tl > EPS).sum()
    return np.float32(total / (npos + EPS))


def _make_in_maps(X: np.ndarray, lab: np.ndarray, blocks, T: int, NT: int):
    B, D = X.shape
    sq = (X.astype(np.float64) ** 2).sum(axis=1).astype(np.float32)
    W = B + 128 + 2 * T

    XT = np.ascontiguousarray(X.T)
    in_maps = []
    for core in range(N_CORES):
        cls_a, cls_b = blocks[core]
        row_idx = np.zeros(128, dtype=np.int64)
        for blk, cls in enumerate((cls_a, cls_b)):
            m = len(cls)
            r0 = blk * RB
            if m:
                row_idx[r0 : r0 + m] = cls
                row_idx[r0 + m : r0 + RB] = cls[0]

        par_idx = np.zeros(2 * T, dtype=np.int64)
        for blk, cls in enumerate((cls_a, cls_b)):
            m = len(cls)
            c0 = blk * T
            if m:
                par_idx[c0 : c0 + m] = cls

        combo = np.empty((D + 1, W), dtype=np.float32)
        combo[:D, :B] = XT
        combo[D, :B] = -0.5 * sq
        combo[:D, B : B + 128] = XT[:, row_idx]
        combo[D, B : B + 128] = 1.0
        combo[:D, B + 128 :] = XT[:, par_idx]
        combo[D, B + 128 :] = -0.5 * sq[par_idx]

        combo_hi = combo.astype(NP_BF16)
        lo = (combo[D, :] - combo_hi[D, :].astype(np.float32)).astype(NP_BF16)
        aug = np.stack([combo_hi[D], lo])  # [2, W]
        # packed [128, 4, W]: partition p, chunk c -> K-row c*128+p
        packed = np.ascontiguousarray(
            combo_hi[:D].reshape(4, 128, W).transpose(1, 0, 2)
        ).reshape(128, 4 * W)

        sq_a = sq[row_idx].reshape(128, 1).astype(np.float32)

        # -SHIFT on valid columns, +BIG on own-class columns (incl. self)
        m_in = np.full((128, B), -SHIFT, dtype=np.float32)
        for blk, cls in enumerate((cls_a, cls_b)):
            if len(cls):
                own = np.isin(lab, lab[cls[0]])
                m_in[blk * RB : (blk + 1) * RB, own] = BIG

        # pair tables: one-hot anchor pick and bias-column select
        onehot = np.zeros((128, NT * 128), dtype=NP_BF16)
        selm = np.zeros((128, NT * 2 * T), dtype=NP_BF16)
        p = 0
        for blk, cls in enumerate((cls_a, cls_b)):
            m = len(cls)
            r0 = blk * RB
            for i in range(m):
                for t in range(m):
                    if t == i:
                        continue
                    tau, q = divmod(p, 128)
                    onehot[r0 + i, tau * 128 + q] = 1.0
                    selm[q, tau * 2 * T + t] = 1.0
                    selm[q, tau * 2 * T + T + t] = 1.0
                    p += 1
        assert p <= NT * 128

        in_maps.append(
            {
                "combo": packed,
                "aug": aug,
                "sq_a": sq_a,
                "m_in": m_in.astype(NP_BF16),
                "onehot": onehot,
                "sel": selm,
                "ident": np.eye(128, dtype=np.float32),
            }
        )
    return in_maps


def kernel(embeddings: np.ndarray, labels: np.ndarray) -> np.ndarray:
    X = np.ascontiguousarray(np.asarray(embeddings), dtype=np.float32)
    lab = np.asarray(labels).astype(np.int64)
    B, D = X.shape
    assert B == B_TOTAL and D == D_MODEL, (B, D)

    sched = _schedule(lab)
    if sched is None:
        return _host_fallback(X, lab)
    blocks, T, NT = sched
    in_maps = _make_in_maps(X, lab, blocks, T, NT)

    key = (B, D, T, NT)
    nc = _PROGRAM_CACHE.get(key)
    if nc is None:
        nc = _build_program(B, D, T, NT)
        _PROGRAM_CACHE[key] = nc

    res = run_bass_kernel_spmd(nc, in_maps, core_ids=list(range(N_CORES)))
    total_sum = 0.0
    total_cnt = 0.0
    for r in res.results:
        o = np.asarray(r["out"], dtype=np.float64)
        total_sum += o[0].sum()
        total_cnt += o[1].sum()
    return np.float32(total_sum / (total_cnt + EPS))


# revision 28
# speedup vs baseline: 1.0705x; 1.0270x over previous
"""Batch-all triplet loss on 8 Trainium2 NeuronCores (Bass/Tile).

Math: with d = pairwise euclidean distance matrix of the B embeddings,
  loss = sum_{i,j,k valid} relu(d[i,j] - d[i,k] + margin) / (#positive + eps)
valid <=> i != j, labels[i] == labels[j], labels[i] != labels[k]
(the other distinctness constraints are implied by the label ones).

Sharding: anchors are grouped by class; each core hosts 2 classes in two
64-row blocks (data-driven gathers keep the single SPMD program uniform).

Per core, on device:
  prep:
  - bf16 matmul of the gathered anchors against all of X^T (row 513
    carries -0.5*||x_j||^2 as a bf16 hi + bf16 lo pair so the squared
    norm stays ~f32 exact) -> ACT Relu(-2*psum + sq_a) -> ACT Sqrt
    -> + mask (-32 shift on valid columns, +1e30 on own-class columns)
    gives d_in (bf16) = this core's rows of the distance matrix.
  - a small partner matmul the same way gives bias[r,t] = d(anchor_r,
    t-th member of r's class) + margin - 32 (bf16), stored next to d_in.
  pair loop (the B^3 work), NT tiles of 128 (anchor,positive) pairs:
  - PE replicates each pair's anchor row (+ its bias row) with a one-hot
    matmul into PSUM: rep[p, 0:640] = d_in[anchor(p), :],
    rep[p, 640:640+T] = bias[anchor(p), :].
  - DVE extracts the pair's bias: tensor_tensor_reduce(rep-bias-cols *
    one-hot-sel) -> bias_pair[p] (f32 scalar per partition).
  - ACT: activation(Relu, in=rep, scale=-1, bias=bias_pair, accum_out)
    = sum_k relu(p - n) for the 128 pairs at once.
  - DVE: tensor_scalar(is_lt, reduce-add) counts n < p (the > eps
    boundary is unreachable at fp granularity, so is_lt is exact).
  - invalid negatives contribute 0 (the +1e30 mask), padded pairs are
    all-zero rows with bias 0 and contribute 0 to both sums.
  - free-dim reduce -> [128, 2] stats DMA'd out; host adds them up.
"""

import numpy as np

import bass_rust
import concourse.bass as bass
import concourse.tile as tile
from concourse import mybir
from concourse.bass_utils import run_bass_kernel_spmd

N_CORES = 8
D_MODEL = 512
B_TOTAL = 640
MARGIN = 0.3
EPS = 1e-8
RB = 64  # rows per block (max class size the device path supports)
BIG = 1.0e30
SHIFT = 32.0  # d values live in ~[27.6, 37]; recentring helps bf16
F32 = mybir.dt.float32
BF16 = mybir.dt.bfloat16
NP_BF16 = mybir.dt.np(mybir.dt.bfloat16)

_PROGRAM_CACHE: dict = {}


def _split_multi_waits(nc):
    """This toolchain's walrus codegen supports only ONE sync-wait per
    instruction; Tile can emit several. Move the extra waits onto
    same-engine NoOps inserted immediately before the instruction."""
    for func in nc.m.functions:
        for block in func.blocks:
            out = []
            for inst in block.instructions:
                si = inst.sync_info
                waits = list(si.on_wait) if si else []
                if len(waits) > 1:
                    for j, w in enumerate(waits[:-1]):
                        nop = mybir.InstNoOp(
                            name=f"{inst.name}-wsplit{j}", ins=[], outs=[]
                        )
                        nop.engine = inst.engine
                        nop.sync_info = bass_rust.SyncInfo(on_wait=[w], on_update=[])
                        out.append(nop)
                    inst.sync_info = bass_rust.SyncInfo(
                        on_wait=[waits[-1]], on_update=list(si.on_update)
                    )
                out.append(inst)
            block.instructions = out


def _build_program(B: int, D: int, T: int, NT: int):
    """One SPMD program for all 8 cores; per-core behavior is data-driven."""
    nc = bass.Bass()

    # combo columns: [0:B) moving operand (X^T), [B:B+128) stationary
    # anchor gather, [B+128:B+128+2T) positive-partner gather. Packed
    # DRAM layout [128, 4, W] so one DMA covers all four K=128 chunks
    # with long per-partition descriptors; row 512 (hi) + its bf16
    # residual live in `aug` ([2, W]).
    W = B + 128 + 2 * T
    combo = nc.declare_dram_parameter("combo", [128, 4 * W], BF16, isOutput=False)
    aug = nc.declare_dram_parameter("aug", [2, W], BF16, isOutput=False)
    sq_a = nc.declare_dram_parameter("sq_a", [128, 1], F32, isOutput=False)
    m_in = nc.declare_dram_parameter("m_in", [128, B], BF16, isOutput=False)
    onehot = nc.declare_dram_parameter("onehot", [128, NT * 128], BF16, isOutput=False)
    sel = nc.declare_dram_parameter("sel", [128, NT * 2 * T], BF16, isOutput=False)
    out_d = nc.declare_dram_parameter("out", [2, 128], F32, isOutput=True)

    WR = B + 2 * T  # replicated tile: negatives + bias hi + bias lo columns

    with tile.TileContext(nc) as tc:
        with (
            tc.tile_pool(name="const", bufs=1) as const,
            tc.tile_pool(name="work", bufs=1) as work,
        ):
            # preload the ACT table set while DMAs run
            warm = const.tile([1, 8], F32)
            nc.vector.memset(warm, 1.0)
            nc.scalar.activation(out=warm, in_=warm,
                                 func=mybir.ActivationFunctionType.Relu)
            nc.scalar.activation(out=warm, in_=warm,
                                 func=mybir.ActivationFunctionType.Sqrt)
            ktile = const.tile([128, 4, W], BF16)
            combo3 = combo[:, :].rearrange("p (c w) -> p c w", c=4)
            for ki in range(4):
                nc.sync.dma_start(out=ktile[:, ki, :], in_=combo3[:, ki, :])
            taug_hi = const.tile([1, W], BF16)
            nc.sync.dma_start(out=taug_hi, in_=aug[0:1, :])
            taug_lo = const.tile([1, W], BF16)
            nc.sync.dma_start(out=taug_lo, in_=aug[1:2, :])
            sqa = const.tile([128, 1], F32)
            nc.sync.dma_start(out=sqa, in_=sq_a[:, :])
            minm = const.tile([128, B], BF16)
            nc.sync.dma_start(out=minm, in_=m_in[:, :])
            t_oh = const.tile([128, NT * 128], BF16)
            ohq = (NT + 3) // 4 * 128
            for c in range(4):
                c0, c1 = c * ohq, min((c + 1) * ohq, NT * 128)
                if c0 < c1:
                    nc.sync.dma_start(out=t_oh[:, c0:c1], in_=onehot[:, c0:c1])
            t_sel = const.tile([128, NT * 2 * T], BF16)
            sq2 = (NT + 1) // 2 * 2 * T
            for c in range(2):
                c0, c1 = c * sq2, min((c + 1) * sq2, NT * 2 * T)
                if c0 < c1:
                    nc.sync.dma_start(out=t_sel[:, c0:c1], in_=sel[:, c0:c1])
            ident = nc.declare_dram_parameter("ident", [128, 128], F32, isOutput=False)
            t_id = const.tile([128, 128], F32)
            nc.sync.dma_start(out=t_id, in_=ident[:, :])

            L0, L1 = B, B + 128  # stationary (anchor) columns
            P0 = B + 128  # partner columns

            rhs_c = work.tile([128, WR], BF16)  # [d_in | bias] combined

            with tc.tile_pool(name="psum_prep", bufs=1, space="PSUM") as psp:
                # ---- distance rows: PSUM g = dot(x_a, x_j) - 0.5*sq_j ----
                g = psp.tile([128, B], F32)
                for n0, n1 in [(0, 512), (512, B)]:
                    for ki in range(4):
                        nc.tensor.matmul(
                            g[:, n0:n1],
                            ktile[:, ki, L0:L1],
                            ktile[:, ki, n0:n1],
                            start=(ki == 0),
                            stop=False,
                        )
                    nc.tensor.matmul(
                        g[:, n0:n1], taug_hi[:, L0:L1], taug_hi[:, n0:n1],
                        start=False, stop=False,
                    )
                    nc.tensor.matmul(
                        g[:, n0:n1], taug_hi[:, L0:L1], taug_lo[:, n0:n1],
                        start=False, stop=True,
                    )
                dsq = work.tile([128, B], F32)
                nc.scalar.activation(
                    out=dsq, in_=g, func=mybir.ActivationFunctionType.Relu,
                    bias=sqa, scale=-2.0,
                )
                dmat = work.tile([128, B], F32)
                nc.scalar.activation(
                    out=dmat, in_=dsq, func=mybir.ActivationFunctionType.Sqrt,
                )
                # d_in = d - 32 on valid columns, +1e30 on own-class cols
                nc.vector.tensor_add(rhs_c[:, 0:B], dmat, minm)

                # ---- positive-pair distances: bias[r, t] ----
                pb = psp.tile([128, T], F32)
                for blk in range(2):
                    r0, r1 = blk * RB, (blk + 1) * RB
                    c0, c1 = P0 + blk * T, P0 + (blk + 1) * T
                    for ki in range(4):
                        nc.tensor.matmul(
                            pb[r0:r1, :],
                            ktile[:, ki, L0 + r0 : L0 + r1],
                            ktile[:, ki, c0:c1],
                            start=(ki == 0),
                            stop=False,
                        )
                    nc.tensor.matmul(
                        pb[r0:r1, :], taug_hi[:, L0 + r0 : L0 + r1],
                        taug_hi[:, c0:c1], start=False, stop=False,
                    )
                    nc.tensor.matmul(
                        pb[r0:r1, :], taug_hi[:, L0 + r0 : L0 + r1],
                        taug_lo[:, c0:c1], start=False, stop=True,
                    )
                bsq = work.tile([128, T], F32)
                nc.scalar.activation(
                    out=bsq, in_=pb, func=mybir.ActivationFunctionType.Relu,
                    bias=sqa, scale=-2.0,
                )
                bd = work.tile([128, T], F32)
                nc.scalar.activation(
                    out=bd, in_=bsq, func=mybir.ActivationFunctionType.Sqrt,
                )
                # bias = d_pos + margin - 32, carried as bf16 hi + lo so
                # the pair threshold stays ~f32-exact (avoids bf16 grid
                # ties between bias and distances, which bias the count)
                bias_f = work.tile([128, T], F32)
                nc.vector.tensor_scalar(
                    out=bias_f, in0=bd,
                    scalar1=float(MARGIN - SHIFT), scalar2=None,
                    op0=mybir.AluOpType.add,
                )
                nc.vector.tensor_copy(rhs_c[:, B : B + T], bias_f)
                nc.vector.tensor_sub(
                    rhs_c[:, B + T : B + 2 * T], bias_f, rhs_c[:, B : B + T]
                )

            # ---- pair loop ----
            with (
                tc.tile_pool(name="psum_loop", bufs=3, space="PSUM") as psl,
                tc.tile_pool(name="psum_acc", bufs=1, space="PSUM") as psa,
                tc.tile_pool(name="scr_a", bufs=2) as scr_a,
                tc.tile_pool(name="scr_v", bufs=2) as scr_v,
                tc.tile_pool(name="bp", bufs=3) as bpp,
            ):
                sum_ps = psa.tile([128, NT], F32)
                cnt_cols = work.tile([128, NT], F32)
                for tau in range(NT):
                    rep = psl.tile([128, WR], F32, tag="rep")
                    oh = t_oh[:, tau * 128 : (tau + 1) * 128]
                    for n0, n1 in [(0, 512), (512, WR)]:
                        nc.tensor.matmul(
                            rep[:, n0:n1], oh, rhs_c[:, n0:n1],
                            start=True, stop=True,
                        )
                    bias_p = bpp.tile([128, 1], F32, tag="bias_p")
                    ttr_o = scr_v.tile([128, 2 * T], F32, tag="ttr")
                    nc.vector.scalar_tensor_tensor(
                        out=ttr_o, in0=rep[:, B : B + 2 * T], scalar=1.0,
                        in1=t_sel[:, tau * 2 * T : (tau + 1) * 2 * T],
                        op0=mybir.AluOpType.mult, op1=mybir.AluOpType.mult,
                        accum_out=bias_p,
                    )
                    o1 = scr_a.tile([128, B], F32, tag="oact")
                    nc.scalar.activation(
                        out=o1, in_=rep[:, 0:B],
                        func=mybir.ActivationFunctionType.Relu,
                        bias=bias_p, scale=-1.0,
                        accum_out=sum_ps[:, tau : tau + 1],
                    )
                    o2 = scr_v.tile([128, B], F32, tag="ocnt")
                    nc.vector.tensor_scalar(
                        out=o2, in0=rep[:, 0:B],
                        scalar1=bias_p, scalar2=0.0,
                        op0=mybir.AluOpType.is_lt, op1=mybir.AluOpType.add,
                        accum_out=cnt_cols[:, tau : tau + 1],
                    )

                # ---- free-dim reduce; transpose so the out DMA is two
                # long descriptors instead of 128 8-byte ones ----
                stat = work.tile([128, 2], F32)
                nc.vector.tensor_reduce(
                    out=stat[:, 0:1], in_=sum_ps, axis=mybir.AxisListType.X,
                    op=mybir.AluOpType.add,
                )
                nc.vector.tensor_reduce(
                    out=stat[:, 1:2], in_=cnt_cols, axis=mybir.AxisListType.X,
                    op=mybir.AluOpType.add,
                )
                stat_t = psa.tile([2, 128], F32, tag="stat_t")
                nc.tensor.transpose(stat_t, stat, t_id)
                stat_s = work.tile([2, 128], F32)
                nc.vector.tensor_copy(stat_s, stat_t)
                nc.sync.dma_start(out=out_d[:, :], in_=stat_s)

    _split_multi_waits(nc)
    return nc


def _schedule(labels: np.ndarray):
    """Group anchors by class, pair classes onto cores (big with small)."""
    vals, counts = np.unique(labels, return_counts=True)
    classes = [np.nonzero(labels == v)[0] for v in vals]
    order = np.argsort(-counts, kind="stable")
    classes = [classes[i] for i in order]
    sizes = [len(c) for c in classes]
    if len(classes) > 2 * N_CORES or max(sizes) > RB:
        return None  # device path infeasible for this label layout
    while len(classes) < 2 * N_CORES:
        classes.append(np.zeros((0,), dtype=np.int64))
    blocks = []
    for i in range(N_CORES):
        blocks.append((classes[i], classes[2 * N_CORES - 1 - i]))
    T = max(1, max(len(c) for c, _ in blocks))
    npairs = [len(a) * (len(a) - 1) + len(b) * (len(b) - 1) for a, b in blocks]
    NT = max(1, (max(npairs) + 127) // 128)
    return blocks, T, NT


def _host_fallback(X: np.ndarray, labels: np.ndarray) -> np.float32:
    """Exact numpy implementation (only for label layouts the device
    schedule cannot represent — cannot occur for randint(0,16) labels)."""
    Xd = X.astype(np.float64)
    dot = Xd @ Xd.T
    sq = np.diag(dot).copy()
    dm = np.maximum(sq[None, :] - 2.0 * dot + sq[:, None], 0.0)
    zero = dm == 0.0
    dm = np.sqrt(dm + zero * EPS) * (1.0 - zero)
    total = 0.0
    npos = 0
    B = len(labels)
    for i in range(B):
        pos = (labels == labels[i]) & (np.arange(B) != i)
        neg = labels != labels[i]
        p = dm[i, pos] + MARGIN
        n = dm[i, neg]
        tl = np.maximum(p[:, None] - n[None, :], 0.0)
        total += tl.sum()
        npos += (tl > EPS).sum()
    return np.float32(total / (npos + EPS))


def _make_in_maps(X: np.ndarray, lab: np.ndarray, blocks, T: int, NT: int):
    B, D = X.shape
    sq = (X.astype(np.float64) ** 2).sum(axis=1).astype(np.float32)
    W = B + 128 + 2 * T

    XT = np.ascontiguousarray(X.T)
    in_maps = []
    for core in range(N_CORES):
        cls_a, cls_b = blocks[core]
        row_idx = np.zeros(128, dtype=np.int64)
        for blk, cls in enumerate((cls_a, cls_b)):
            m = len(cls)
            r0 = blk * RB
            if m:
                row_idx[r0 : r0 + m] = cls
                row_idx[r0 + m : r0 + RB] = cls[0]

        par_idx = np.zeros(2 * T, dtype=np.int64)
        for blk, cls in enumerate((cls_a, cls_b)):
            m = len(cls)
            c0 = blk * T
            if m:
                par_idx[c0 : c0 + m] = cls

        combo = np.empty((D + 1, W), dtype=np.float32)
        combo[:D, :B] = XT
        combo[D, :B] = -0.5 * sq
        combo[:D, B : B + 128] = XT[:, row_idx]
        combo[D, B : B + 128] = 1.0
        combo[:D, B + 128 :] = XT[:, par_idx]
        combo[D, B + 128 :] = -0.5 * sq[par_idx]

        combo_hi = combo.astype(NP_BF16)
        lo = (combo[D, :] - combo_hi[D, :].astype(np.float32)).astype(NP_BF16)
        aug = np.stack([combo_hi[D], lo])  # [2, W]
        # packed [128, 4, W]: partition p, chunk c -> K-row c*128+p
        packed = np.ascontiguousarray(
            combo_hi[:D].reshape(4, 128, W).transpose(1, 0, 2)
        ).reshape(128, 4 * W)

        sq_a = sq[row_idx].reshape(128, 1).astype(np.float32)

        # -SHIFT on valid columns, +BIG on own-class columns (incl. self)
        m_in = np.full((128, B), -SHIFT, dtype=np.float32)
        for blk, cls in enumerate((cls_a, cls_b)):
            if len(cls):
                own = np.isin(lab, lab[cls[0]])
                m_in[blk * RB : (blk + 1) * RB, own] = BIG

        # pair tables: one-hot anchor pick and bias-column select
        onehot = np.zeros((128, NT * 128), dtype=NP_BF16)
        selm = np.zeros((128, NT * 2 * T), dtype=NP_BF16)
        p = 0
        for blk, cls in enumerate((cls_a, cls_b)):
            m = len(cls)
            r0 = blk * RB
            for i in range(m):
                for t in range(m):
                    if t == i:
                        continue
                    tau, q = divmod(p, 128)
                    onehot[r0 + i, tau * 128 + q] = 1.0
                    selm[q, tau * 2 * T + t] = 1.0
                    selm[q, tau * 2 * T + T + t] = 1.0
                    p += 1
        assert p <= NT * 128

        in_maps.append(
            {
                "combo": packed,
                "aug": aug,
                "sq_a": sq_a,
                "m_in": m_in.astype(NP_BF16),
                "onehot": onehot,
                "sel": selm,
                "ident": np.eye(128, dtype=np.float32),
            }
        )
    return in_maps


def kernel(embeddings: np.ndarray, labels: np.ndarray) -> np.ndarray:
    X = np.ascontiguousarray(np.asarray(embeddings), dtype=np.float32)
    lab = np.asarray(labels).astype(np.int64)
    B, D = X.shape
    assert B == B_TOTAL and D == D_MODEL, (B, D)

    sched = _schedule(lab)
    if sched is None:
        return _host_fallback(X, lab)
    blocks, T, NT = sched
    in_maps = _make_in_maps(X, lab, blocks, T, NT)

    key = (B, D, T, NT)
    nc = _PROGRAM_CACHE.get(key)
    if nc is None:
        nc = _build_program(B, D, T, NT)
        _PROGRAM_CACHE[key] = nc

    res = run_bass_kernel_spmd(nc, in_maps, core_ids=list(range(N_CORES)))
    total_sum = 0.0
    total_cnt = 0.0
    for r in res.results:
        o = np.asarray(r["out"], dtype=np.float64)
        total_sum += o[0].sum()
        total_cnt += o[1].sum()
    return np.float32(total_sum / (total_cnt + EPS))
